# revision 2
# baseline (speedup 1.0000x reference)
"""AttentivePooling Trainium2 kernel (8 NeuronCores, batch-parallel SPMD).

kernel(**inputs) takes the FULL unsharded inputs (numpy), shards batch-wise
across 8 cores, runs a Bass/Tile kernel per core, and returns the FULL
(16, 10240) float32 output.

Per-core pipeline (2 batches each, natural layout: channels on partitions,
time on the free axis):
  phase 1: stream x (bf16) tiles; TensorE accumulates h = W1x @ x in PSUM;
           DVE accumulates sum_t x, ACT accumulates sum_t x^2 (per channel).
  phase 2: finalize gmean/gstd; TensorE mat-vec v = Wg @ [gmean; gstd] + b1.
  phase 3: ACT: relu(h + v) -> tanh(bn_scale * u + bn_bias) -> h_t (bf16).
  phase 4: per channel tile: TensorE logits = W2 @ h_t; ACT exp(logits + b2)
           with accumulated Z; DVE fused multiply+reduce for S1 = sum x*e and
           S2 = sum x^2*e.
  phase 5: mu = S1/Z, rh = sqrt(clip(S2/Z - mu^2, 1e-5)); DMA out.
"""
import contextlib
import sys

for _p in ("/opt/trn_rl_repo",):
    if _p not in sys.path:
        sys.path.insert(0, _p)

import numpy as np
import ml_dtypes

import concourse.bacc as bacc
import concourse.tile as tile
from concourse import mybir

BL = 2          # batches per core
NCORES = 8
B = BL * NCORES
C = 5120
T = 1024
CR = 256
NCT = C // 128
NGK = 2 * C // 128
BF16 = mybir.dt.bfloat16
F32 = mybir.dt.float32
NP_BF16 = ml_dtypes.bfloat16
ALU = mybir.AluOpType
ACTF = mybir.ActivationFunctionType


def _build():
    nc = bacc.Bacc("TRN2", target_bir_lowering=False, num_devices=NCORES)

    x_ext = nc.dram_tensor("x", [BL, C, T], BF16, kind="ExternalInput").ap()
    w1xT_ext = nc.dram_tensor("w1xT", [128, NCT, CR], BF16, kind="ExternalInput").ap()
    wgT_ext = nc.dram_tensor("wgT", [128, NGK, CR], BF16, kind="ExternalInput").ap()
    w2T_ext = nc.dram_tensor("w2T", [128, 2, NCT, 128], BF16, kind="ExternalInput").ap()
    b1_ext = nc.dram_tensor("b1r", [1, CR], F32, kind="ExternalInput").ap()
    bn_ext = nc.dram_tensor("bnaff", [2, CR], F32, kind="ExternalInput").ap()
    b2_ext = nc.dram_tensor("b2t", [128, NCT], F32, kind="ExternalInput").ap()
    out_ext = nc.dram_tensor("out", [BL, 2 * C], F32, kind="ExternalOutput").ap()

    with tile.TileContext(nc) as tc:
        with contextlib.ExitStack() as ctx:
            singles = ctx.enter_context(tc.tile_pool(name="singles", bufs=1))
            dramp = ctx.enter_context(tc.tile_pool(name="dramp", bufs=2, space="DRAM"))
            xp = ctx.enter_context(tc.tile_pool(name="xp", bufs=20))
            ep = ctx.enter_context(tc.tile_pool(name="ep", bufs=6))
            scr = ctx.enter_context(tc.tile_pool(name="scr", bufs=2))
            stats = ctx.enter_context(tc.tile_pool(name="stats", bufs=2))
            htp = ctx.enter_context(tc.tile_pool(name="htp", bufs=2))
            psum = ctx.enter_context(tc.tile_pool(name="psum", bufs=2, space="PSUM"))
            psum_l = ctx.enter_context(tc.tile_pool(name="psum_l", bufs=2, space="PSUM"))

            w1xt = singles.tile([128, NCT, CR], BF16)
            nc.sync.dma_start(out=w1xt[:, :, :], in_=w1xT_ext[:, :, :])
            wgt = singles.tile([128, NGK, CR], BF16)
            nc.sync.dma_start(out=wgt[:, :, :], in_=wgT_ext[:, :, :])
            w2t = singles.tile([128, 2, NCT, 128], BF16)
            nc.sync.dma_start(out=w2t[:, :, :, :], in_=w2T_ext[:, :, :, :])
            b1r = singles.tile([1, CR], F32)
            nc.sync.dma_start(out=b1r[:, :], in_=b1_ext[:, :])
            bncol = singles.tile([128, 2, 2], F32)
            for half in range(2):
                nc.sync.dma_start(
                    out=bncol[:, half, :],
                    in_=bn_ext[:, half * 128:(half + 1) * 128].rearrange("a p -> p a"))
            b2t = singles.tile([128, NCT], F32)
            nc.sync.dma_start(out=b2t[:, :], in_=b2_ext[:, :])

            def batch_body(b):
                h_ps = [psum.tile([128, T], F32, tag="hps", name=f"hps{b}_{i}")
                        for i in range(2)]
                sx = stats.tile([128, NCT], F32, tag="sx")
                sxsq = stats.tile([128, NCT], F32, tag="sxsq")
                scrv = scr.tile([128, T], BF16, tag="scrv")
                scra = scr.tile([128, T], BF16, tag="scra")
                for ct in range(NCT):
                    xt = xp.tile([128, T], BF16, tag="xt1")
                    nc.sync.dma_start(out=xt[:, :],
                                      in_=x_ext[b, ct * 128:(ct + 1) * 128, :])
                    for mh in range(2):
                        for nchunk in range(2):
                            nc.tensor.matmul(
                                h_ps[mh][:, nchunk * 512:(nchunk + 1) * 512],
                                lhsT=w1xt[:, ct, mh * 128:(mh + 1) * 128],
                                rhs=xt[:, nchunk * 512:(nchunk + 1) * 512],
                                start=(ct == 0), stop=(ct == NCT - 1))
                    nc.vector.tensor_scalar(
                        out=scrv[:, :], in0=xt[:, :], scalar1=1.0, scalar2=0.0,
                        op0=ALU.mult, op1=ALU.add, accum_out=sx[:, ct:ct + 1])
                    nc.scalar.activation(
                        out=scra[:, :], in_=xt[:, :], func=ACTF.Square,
                        accum_out=sxsq[:, ct:ct + 1])

                gm = stats.tile([128, NCT], F32, tag="gm")
                gs = stats.tile([128, NCT], F32, tag="gs")
                tmp = stats.tile([128, NCT], F32, tag="tmp")
                nc.vector.tensor_scalar(out=gm[:, :], in0=sx[:, :], scalar1=1.0 / T,
                                        scalar2=0.0, op0=ALU.mult, op1=ALU.add)
                nc.vector.scalar_tensor_tensor(
                    out=tmp[:, :], in0=sx[:, :], scalar=1.0 / T, in1=sx[:, :],
                    op0=ALU.mult, op1=ALU.mult)
                nc.vector.tensor_tensor(out=tmp[:, :], in0=sxsq[:, :], in1=tmp[:, :],
                                        op=ALU.subtract)
                nc.vector.tensor_scalar(out=tmp[:, :], in0=tmp[:, :],
                                        scalar1=1.0 / (T - 1), scalar2=0.0,
                                        op0=ALU.mult, op1=ALU.add)
                nc.scalar.activation(out=gs[:, :], in_=tmp[:, :], func=ACTF.Sqrt)
                gmb = stats.tile([128, NCT], BF16, tag="gmb")
                gsb = stats.tile([128, NCT], BF16, tag="gsb")
                nc.vector.tensor_copy(gmb[:, :], gm[:, :])
                nc.vector.tensor_copy(gsb[:, :], gs[:, :])

                v_ps = psum_l.tile([1, CR], F32, tag="lps", name=f"vps{b}")
                for gk in range(NGK):
                    g_col = (gmb[:, gk:gk + 1] if gk < NCT
                             else gsb[:, gk - NCT:gk - NCT + 1])
                    nc.tensor.matmul(v_ps[:, :], lhsT=g_col, rhs=wgt[:, gk, :],
                                     start=(gk == 0), stop=(gk == NGK - 1))
                vrow = stats.tile([1, CR], F32, tag="vrow")
                nc.vector.tensor_tensor(out=vrow[:, :], in0=v_ps[:, :], in1=b1r[:, :],
                                        op=ALU.add)
                vscr = dramp.tile([1, CR], F32, tag="vscr")
                nc.sync.dma_start(out=vscr[:, :], in_=vrow[:, :])
                vcol = stats.tile([128, 2], F32, tag="vcol")
                nc.sync.dma_start(
                    out=vcol[:, :],
                    in_=vscr[0, :].rearrange("(half p) -> p half", p=128))

                ht = [htp.tile([128, T], BF16, tag="ht", name=f"ht{b}_{i}")
                      for i in range(2)]
                for mh in range(2):
                    u = scr.tile([128, T], BF16, tag="u")
                    nc.scalar.activation(out=u[:, :], in_=h_ps[mh][:, :],
                                         func=ACTF.Relu,
                                         bias=vcol[:, mh:mh + 1], scale=1.0)
                    nc.scalar.activation(out=ht[mh][:, :], in_=u[:, :], func=ACTF.Tanh,
                                         bias=bncol[:, mh, 1:2], scale=bncol[:, mh, 0:1])

                zz = stats.tile([128, NCT], F32, tag="zz")
                s1 = stats.tile([128, NCT], F32, tag="s1")
                s2 = stats.tile([128, NCT], F32, tag="s2")
                for ct in range(NCT):
                    xt4 = xp.tile([128, T], BF16, tag="xt4")
                    nc.sync.dma_start(out=xt4[:, :],
                                      in_=x_ext[b, ct * 128:(ct + 1) * 128, :])
                    l_ps = psum_l.tile([128, T], F32, tag="lps")
                    for oh in range(2):
                        for nchunk in range(2):
                            nc.tensor.matmul(
                                l_ps[:, nchunk * 512:(nchunk + 1) * 512],
                                lhsT=w2t[:, oh, ct, :],
                                rhs=ht[oh][:, nchunk * 512:(nchunk + 1) * 512],
                                start=(oh == 0), stop=(oh == 1))
                    et = ep.tile([128, T], BF16, tag="et")
                    nc.scalar.activation(out=et[:, :], in_=l_ps[:, :], func=ACTF.Exp,
                                         bias=b2t[:, ct:ct + 1], scale=1.0,
                                         accum_out=zz[:, ct:ct + 1])
                    pt = scr.tile([128, T], BF16, tag="pt")
                    nc.vector.scalar_tensor_tensor(
                        out=pt[:, :], in0=xt4[:, :], scalar=1.0, in1=et[:, :],
                        op0=ALU.mult, op1=ALU.mult, accum_out=s1[:, ct:ct + 1])
                    qt = scr.tile([128, T], BF16, tag="qt")
                    nc.vector.scalar_tensor_tensor(
                        out=qt[:, :], in0=pt[:, :], scalar=1.0, in1=xt4[:, :],
                        op0=ALU.mult, op1=ALU.mult, accum_out=s2[:, ct:ct + 1])

                rz = stats.tile([128, NCT], F32, tag="rz")
                mu = stats.tile([128, NCT], F32, tag="mu")
                rh = stats.tile([128, NCT], F32, tag="rh")
                t2 = stats.tile([128, NCT], F32, tag="t2")
                msq = stats.tile([128, NCT], F32, tag="msq")
                nc.vector.reciprocal(out=rz[:, :], in_=zz[:, :])
                nc.vector.tensor_tensor(out=mu[:, :], in0=s1[:, :], in1=rz[:, :],
                                        op=ALU.mult)
                nc.vector.tensor_tensor(out=t2[:, :], in0=s2[:, :], in1=rz[:, :],
                                        op=ALU.mult)
                nc.vector.tensor_tensor(out=msq[:, :], in0=mu[:, :], in1=mu[:, :],
                                        op=ALU.mult)
                nc.vector.tensor_tensor(out=t2[:, :], in0=t2[:, :], in1=msq[:, :],
                                        op=ALU.subtract)
                nc.vector.tensor_scalar(out=t2[:, :], in0=t2[:, :], scalar1=1e-5,
                                        scalar2=0.0, op0=ALU.max, op1=ALU.add)
                nc.scalar.activation(out=rh[:, :], in_=t2[:, :], func=ACTF.Sqrt)
                nc.sync.dma_start(
                    out=out_ext[b, 0:C].rearrange("(ct p) -> p ct", p=128),
                    in_=mu[:, :])
                nc.sync.dma_start(
                    out=out_ext[b, C:2 * C].rearrange("(ct p) -> p ct", p=128),
                    in_=rh[:, :])

            for b in range(BL):
                batch_body(b)

    nc.compile()
    return nc


def _host_prep(x, w1, b1, gamma, beta, run_mean, run_var, w2, b2):
    w1xT = np.ascontiguousarray(
        w1[:, :C].reshape(CR, NCT, 128).transpose(2, 1, 0)).astype(NP_BF16)
    wgT = np.ascontiguousarray(
        w1[:, C:].reshape(CR, NGK, 128).transpose(2, 1, 0)).astype(NP_BF16)
    w2T = np.ascontiguousarray(
        w2.reshape(NCT, 128, 2, 128).transpose(3, 2, 0, 1)).astype(NP_BF16)
    inv = gamma / np.sqrt(run_var + 1e-5)
    bnaff = np.stack([inv, beta - run_mean * inv]).astype(np.float32)
    b1r = b1.reshape(1, CR).astype(np.float32)
    b2t = np.ascontiguousarray(b2.reshape(NCT, 128).T).astype(np.float32)

    xb = x.astype(NP_BF16)
    in_maps = []
    for core in range(NCORES):
        in_maps.append({
            "x": np.ascontiguousarray(xb[core * BL:(core + 1) * BL]),
            "w1xT": w1xT, "wgT": wgT, "w2T": w2T,
            "b1r": b1r, "bnaff": bnaff, "b2t": b2t,
        })
    return in_maps


_NC_CACHE = []


def kernel(x, w1, b1, gamma, beta, run_mean, run_var, w2, b2):
    x = np.asarray(x, np.float32)
    w1 = np.asarray(w1, np.float32)
    b1 = np.asarray(b1, np.float32)
    gamma = np.asarray(gamma, np.float32)
    beta = np.asarray(beta, np.float32)
    run_mean = np.asarray(run_mean, np.float32)
    run_var = np.asarray(run_var, np.float32)
    w2 = np.asarray(w2, np.float32)
    b2 = np.asarray(b2, np.float32)

    if not _NC_CACHE:
        _NC_CACHE.append(_build())
    nc = _NC_CACHE[0]

    in_maps = _host_prep(x, w1, b1, gamma, beta, run_mean, run_var, w2, b2)

    from concourse import bass2jax
    results = bass2jax.run_bass_via_pjrt(nc, in_maps, n_cores=NCORES)
    out = np.concatenate([results[c]["out"] for c in range(NCORES)], axis=0)
    return out.astype(np.float32)


if __name__ == "__main__":
    rng = np.random.default_rng(0)
    fake = {
        "x": rng.standard_normal((B, C, T), dtype=np.float32),
        "w1": rng.standard_normal((CR, 3 * C), dtype=np.float32) / np.sqrt(3 * C),
        "b1": rng.standard_normal(CR).astype(np.float32) * 0.01,
        "gamma": rng.uniform(0.5, 1.5, CR).astype(np.float32),
        "beta": rng.standard_normal(CR).astype(np.float32) * 0.01,
        "run_mean": rng.standard_normal(CR).astype(np.float32) * 0.1,
        "run_var": rng.uniform(0.5, 1.5, CR).astype(np.float32),
        "w2": rng.standard_normal((C, CR), dtype=np.float32) / np.sqrt(CR),
        "b2": rng.standard_normal(C).astype(np.float32) * 0.01,
    }
    out = kernel(**fake)
    print("kernel output:", out.shape, out.dtype)
